# revision 23
# baseline (speedup 1.0000x reference)
"""Chamfer loss kernel for Trainium2 (8 NeuronCores, SPMD).

Problem: B=8 batches, predict_pc (B,3,M=8192), gt_pc (B,3,N=8192).
loss = mean_m sqrt(min_n ||p_m-g_n||^2 + eps) + mean_n sqrt(min_m ||.||^2 + eps)

Sharding: data-parallel over B — each of the 8 cores handles one batch
independently and emits partial per-partition sums; the host combines them.

Per-core algorithm. The reference's argmin+gather+recompute equals the min
of the distance matrix itself up to fp rounding, so no argmin/gather is
needed — only min reductions. Two symmetric PE streams so both reductions
are free-axis max-reductions (no cross-partition ops, which the BIR
verifier restricts):
  stream A: vA[m,n] = 2*p.g - g2[n];  min_n d2[m,:] = p2[m] - max_n vA[m,:]
  stream B: vB[n,m] = 2*g.p - p2[m];  min_m d2[:,n] = g2[n] - max_m vB[n,:]
Matmuls run in float32r (1 PE cycle/row vs 4 for fp32) with a hi/lo split
(x = xh + xl, three exact cross terms) for fp32-level accuracy, and produce
-d2 DIRECTLY (p2 and g2 both folded in, K=13).  Because -d2 is small near
its max (no large-term cancellation), the PSUM tiles are converted to bf16
by the otherwise-idle ScalarE (Relu of -1x, which also clamps rounding
negatives), and the min-reductions run on DVE as packed-bf16 2x-mode
tensor_tensor min cascades from SBUF — about half the cost of 1x fp32
tensor_reduce from PSUM, splitting the reduction load across two engines.
"""

import numpy as np

B, D, M, N = 8, 3, 8192, 8192
P = 128  # partitions / tile size
NCHUNK = 512  # matmul free dim (one PSUM bank of fp32)
GROUP = 4  # chunks per PSUM tile (4 banks -> FD=2048 reductions)
EPS = 1e-8

_built = None


def _build():
    import concourse.bass as bass
    import concourse.bacc as bacc
    import concourse.mybir as mybir
    import concourse.tile as tile
    from contextlib import ExitStack

    fp32 = mybir.dt.float32
    nc = bacc.Bacc()
    p_ext = nc.declare_dram_parameter("p", [D, M], fp32, isOutput=False)
    g_ext = nc.declare_dram_parameter("g", [D, N], fp32, isOutput=False)
    negones_ext = nc.declare_dram_parameter(
        "negones", [1, M], mybir.dt.float32r, isOutput=False
    )
    out_ext = nc.declare_dram_parameter("out", [P, 2], fp32, isOutput=True)

    ntiles = M // P  # 64 (same for N)
    ngroups = N // (GROUP * NCHUNK)  # 4 groups of 2048 per tile row

    with ExitStack() as ctx:
        tc = ctx.enter_context(tile.TileContext(nc))
        singles = ctx.enter_context(tc.tile_pool(name="singles", bufs=1))
        scratch = ctx.enter_context(tc.tile_pool(name="scratch", bufs=1))
        psum = ctx.enter_context(tc.tile_pool(name="psum", bufs=2, space="PSUM"))

        # weights-form (2x,2y,2z,1) and rhs-form (x,y,z,-sq) matrices.
        # float32r: PE runs these at 1 cycle/row for N>=256 (vs 4 for fp32);
        # producers must emit fp32r-rounded values (walrus verifier rule).
        fp32r = mybir.dt.float32r
        bf16 = mybir.dt.bfloat16
        pw = singles.tile([13, M], fp32r)
        pr = singles.tile([13, M], fp32r)
        gw = singles.tile([13, N], fp32r)
        gr = singles.tile([13, N], fp32r)
        npairs = ngroups // 2  # cascades run on pairs of 2048-groups
        rowparts = singles.tile([P, ntiles, npairs], fp32)
        colparts = singles.tile([P, ntiles, npairs], fp32)
        ones3 = singles.tile([D, P], fp32)
        nc.vector.memset(ones3, 1.0)
        eps_col = singles.tile([P, 1], fp32)
        nc.vector.memset(eps_col, EPS)
        out_sb = singles.tile([P, 2], fp32)

        SPAN = GROUP * NCHUNK  # 2048: chunk width for staged builds

        def setup_side(ext, w, r, nm):
            """Build the K=13 hi/lo matrices.

            w = (2xh[3], 2xh[3], xl[3], -1, -1, sqh, sql)  (weights form)
            r = (xh[3],  xl[3], 2xh[3], sqh, sql, -1, -1)  (rhs form)
            so that sum_k w_k*r'_k = 2 x.y - y2 - x2 = -d2 exactly to ~fp32.
            Engine ops may only start at partition 0, so rows >=3 are
            filled by DMA from partition-0 staging tiles or row dups.
            """
            raw = scratch.tile([D, M], fp32, tag="raw", name=f"raw_{nm}")
            nc.sync.dma_start(out=raw, in_=ext[:, :])
            # hi parts (fp32r writes round): w[0:3]=2xh (exact 2*round(x)),
            # r[0:3]=xh
            nc.vector.tensor_scalar_mul(w[0:3, :], raw, 2.0)
            nc.vector.tensor_copy(r[0:3, :], raw)
            # dup rows: w[3:6]=2xh, r[6:9]=2xh
            nc.sync.dma_start(out=w[3:6, :], in_=w[0:3, :])
            nc.sync.dma_start(out=r[6:9, :], in_=w[0:3, :])
            # const rows: w[9:11] = -1, r[11:13] = -1
            nc.sync.dma_start(out=w[9:10, :], in_=negones_ext[:, :])
            nc.sync.dma_start(out=w[10:11, :], in_=negones_ext[:, :])
            nc.sync.dma_start(out=r[11:12, :], in_=negones_ext[:, :])
            nc.sync.dma_start(out=r[12:13, :], in_=negones_ext[:, :])
            # lo parts per chunk: xl = round(x - xh) -> w[6:9], r[3:6]
            for ci in range(M // SPAN):
                sl = slice(ci * SPAN, (ci + 1) * SPAN)
                st_xl = scratch.tile(
                    [D, SPAN], fp32r, tag="st_xl", bufs=2, name="st_xl"
                )
                nc.vector.tensor_tensor(
                    out=st_xl, in0=raw[:, sl], in1=r[0:3, sl].bitcast(fp32),
                    op=mybir.AluOpType.subtract,
                )
                nc.sync.dma_start(out=w[6:9, sl], in_=st_xl)
                nc.sync.dma_start(out=r[3:6, sl], in_=st_xl)
            nc.vector.tensor_mul(raw, raw, raw)  # squares, in place
            # sq rows split hi/lo at partition 0 (1024-wide chunks to keep
            # the staging slots small), DMA to r rows 9/10 and w rows 11/12
            SQW = 1024
            for gi in range(M // SQW):
                sl = slice(gi * SQW, (gi + 1) * SQW)
                ps = psum.tile([P, GROUP, NCHUNK], fp32, tag="ps", name="ps_sq")
                for c in range(SQW // NCHUNK):
                    n0 = gi * SQW + c * NCHUNK
                    nc.tensor.matmul(
                        ps[:, c, :], ones3, raw[:, n0 : n0 + NCHUNK],
                        start=True, stop=True,
                    )
                psrow = ps[0:1, 0:2, :].rearrange("p a b -> p (a b)")
                st_sqh = scratch.tile([1, SQW], fp32r, tag="st_sqh", name="st_sqh")
                st_sql = scratch.tile([1, SQW], fp32r, tag="st_sql", name="st_sql")
                nc.scalar.copy(st_sqh, psrow)
                nc.vector.tensor_tensor(
                    out=st_sql, in0=psrow, in1=st_sqh.bitcast(fp32),
                    op=mybir.AluOpType.subtract,
                )
                nc.sync.dma_start(out=r[9:10, sl], in_=st_sqh)
                nc.sync.dma_start(out=r[10:11, sl], in_=st_sql)
                nc.sync.dma_start(out=w[11:12, sl], in_=st_sqh)
                nc.sync.dma_start(out=w[12:13, sl], in_=st_sql)

        setup_side(p_ext, pw, pr, "p")
        setup_side(g_ext, gw, gr, "g")

        # --- main loops: two symmetric streams.  PSUM holds -d2; ScalarE
        # converts each 2048-group to bf16 d2 (Relu of -1x: also clamps
        # fp-rounding negatives), two groups fill a [128, 4096] bf16 buffer,
        # then DVE folds it with packed-bf16 2x tensor_tensor mins + a final
        # 1x reduce.  ACT ~1.85us and DVE ~1.3us per group vs 2.26us DVE-only.
        def stream(w, r, parts):
            for t in range(ntiles):
                lhsT = w[:, t * P : (t + 1) * P]  # [13, 128]
                for pair in range(npairs):
                    bfp = scratch.tile(
                        [P, 2 * SPAN], bf16, tag="st_xl", bufs=2, name="bfp"
                    )
                    for sub in range(2):
                        gi = pair * 2 + sub
                        ps = psum.tile(
                            [P, GROUP, NCHUNK], fp32, tag="ps", name="ps_main"
                        )
                        for c in range(GROUP):
                            n0 = gi * GROUP * NCHUNK + c * NCHUNK
                            nc.tensor.matmul(
                                ps[:, c, :], lhsT, r[:, n0 : n0 + NCHUNK],
                                start=True, stop=True,
                            )
                        nc.scalar.activation(
                            bfp[:, sub * SPAN : (sub + 1) * SPAN],
                            ps.rearrange("p a b -> p (a b)"),
                            mybir.ActivationFunctionType.Relu,
                            scale=-1.0,
                        )
                    c1 = scratch.tile([P, SPAN], bf16, tag="st_sqh", name="c1")
                    nc.vector.tensor_tensor(
                        out=c1, in0=bfp[:, 0:SPAN], in1=bfp[:, SPAN : 2 * SPAN],
                        op=mybir.AluOpType.min,
                    )
                    c2 = scratch.tile([P, SPAN // 2], bf16, tag="st_sql", name="c2")
                    nc.vector.tensor_tensor(
                        out=c2, in0=c1[:, 0 : SPAN // 2], in1=c1[:, SPAN // 2 :],
                        op=mybir.AluOpType.min,
                    )
                    c3 = scratch.tile([P, SPAN // 4], bf16, tag="small", name="c3")
                    nc.vector.tensor_tensor(
                        out=c3, in0=c2[:, 0 : SPAN // 4], in1=c2[:, SPAN // 4 :],
                        op=mybir.AluOpType.min,
                    )
                    nc.vector.tensor_reduce(
                        parts[:, t, pair : pair + 1], c3,
                        axis=mybir.AxisListType.X, op=mybir.AluOpType.min,
                    )

        # B first: its inputs (gw built early in g-setup, pr from p-setup)
        # are ready before gr's staged sq rows, so PE starts sooner.
        stream(gw, pr, colparts)  # vB = 2 g.p - p2 ; max over m
        stream(pw, gr, rowparts)  # vA = 2 p.g - g2 ; max over n

        # --- tails: el = sqrt(min over pairs + eps), per-partition sums ---
        def tail(parts, col):
            d2m = scratch.tile([P, ntiles], fp32, tag="small2", name=f"d2_{col}")
            nc.vector.tensor_reduce(
                d2m, parts, axis=mybir.AxisListType.X, op=mybir.AluOpType.min
            )
            el = scratch.tile([P, ntiles], fp32, tag="small3", name=f"el_{col}")
            nc.scalar.activation(
                el, d2m, mybir.ActivationFunctionType.Sqrt, bias=eps_col
            )
            nc.vector.tensor_reduce(
                out_sb[:, col : col + 1], el,
                axis=mybir.AxisListType.X, op=mybir.AluOpType.add,
            )

        tail(rowparts, 0)
        tail(colparts, 1)

        nc.sync.dma_start(out=out_ext[:, :], in_=out_sb)

    nc.finalize()  # Bacc: compile passes (wait splitting, reg alloc) + freeze
    return nc


def _get_nc():
    global _built
    if _built is None:
        _built = _build()
    return _built


def _run(in_maps, **kw):
    from concourse.bass_utils import run_bass_kernel_spmd

    return run_bass_kernel_spmd(_get_nc(), in_maps, list(range(B)), **kw)


_runner = None


def _get_runner():
    """Cached jitted SPMD executable (run_bass_kernel_spmd rebuilds the jax
    trace on every call, ~0.5s; this keeps the compiled callable alive)."""
    global _runner
    if _runner is not None:
        return _runner
    import jax
    import concourse.mybir as mybir
    from jax.experimental.shard_map import shard_map
    from jax.sharding import Mesh, PartitionSpec
    from concourse import bass2jax

    nc = _get_nc()
    bass2jax.install_neuronx_cc_hook()
    partition_name = nc.partition_id_tensor.name if nc.partition_id_tensor else None
    in_names, out_names, out_avals, zero_shapes = [], [], [], []
    for alloc in nc.m.functions[0].allocations:
        if not isinstance(alloc, mybir.MemoryLocationSet):
            continue
        name = alloc.memorylocations[0].name
        if alloc.kind == "ExternalInput":
            if name != partition_name:
                in_names.append(name)
        elif alloc.kind == "ExternalOutput":
            shape = tuple(alloc.tensor_shape)
            dtype = mybir.dt.np(alloc.dtype)
            out_names.append(name)
            out_avals.append(jax.core.ShapedArray(shape, dtype))
            zero_shapes.append((shape, dtype))
    n_params = len(in_names)
    all_names = in_names + out_names + ([partition_name] if partition_name else [])

    def _body(*args):
        operands = list(args)
        if partition_name is not None:
            operands.append(bass2jax.partition_id_tensor())
        return tuple(
            bass2jax._bass_exec_p.bind(
                *operands,
                out_avals=tuple(out_avals),
                in_names=tuple(all_names),
                out_names=tuple(out_names),
                lowering_input_output_aliases=(),
                sim_require_finite=True,
                sim_require_nnan=True,
                nc=nc,
            )
        )

    devices = jax.devices()[:B]
    mesh = Mesh(np.asarray(devices), ("core",))
    donate = tuple(range(n_params, n_params + len(out_names)))
    sharded = jax.jit(
        shard_map(
            _body,
            mesh=mesh,
            in_specs=(PartitionSpec("core"),) * (n_params + len(out_names)),
            out_specs=(PartitionSpec("core"),) * len(out_names),
            check_rep=False,
        ),
        donate_argnums=donate,
        keep_unused=True,
    )
    _runner = (sharded, in_names, out_names, zero_shapes)
    return _runner


def _run_fast(in_maps):
    sharded, in_names, out_names, zero_shapes = _get_runner()
    concat_in = [
        np.concatenate([np.asarray(in_maps[c][nm]) for c in range(B)], axis=0)
        for nm in in_names
    ]
    concat_zeros = [
        np.zeros((B * s[0], *s[1:]), dt) for (s, dt) in zero_shapes
    ]
    out_arrs = sharded(*concat_in, *concat_zeros)
    outs = []
    for c in range(B):
        d = {}
        for i, nm in enumerate(out_names):
            s, dt = zero_shapes[i]
            d[nm] = np.asarray(out_arrs[i]).reshape(B, *s)[c]
        outs.append(d)
    return outs


def kernel(predict_pc, gt_pc):
    predict_pc = np.asarray(predict_pc, dtype=np.float32)
    gt_pc = np.asarray(gt_pc, dtype=np.float32)
    negones = np.full((1, M), -1.0, dtype=np.float32)
    in_maps = [
        {
            "p": np.ascontiguousarray(predict_pc[i]),
            "g": np.ascontiguousarray(gt_pc[i]),
            "negones": negones,
        }
        for i in range(B)
    ]
    results = _run_fast(in_maps)
    total = 0.0
    for r in results:
        o = np.asarray(r["out"], dtype=np.float64)
        total += o[:, 0].sum() / (B * M) + o[:, 1].sum() / (B * N)
    return np.float32(total)


# revision 24
# speedup vs baseline: 1.0827x; 1.0827x over previous
"""Chamfer loss kernel for Trainium2 (8 NeuronCores, SPMD).

Problem: B=8 batches, predict_pc (B,3,M=8192), gt_pc (B,3,N=8192).
loss = mean_m sqrt(min_n ||p_m-g_n||^2 + eps) + mean_n sqrt(min_m ||.||^2 + eps)

Sharding: data-parallel over B — each of the 8 cores handles one batch
independently and emits partial per-partition sums; the host combines them.

Per-core algorithm. The reference's argmin+gather+recompute equals the min
of the distance matrix itself up to fp rounding, so no argmin/gather is
needed — only min reductions. Two symmetric PE streams so both reductions
are free-axis max-reductions (no cross-partition ops, which the BIR
verifier restricts):
  stream A: vA[m,n] = 2*p.g - g2[n];  min_n d2[m,:] = p2[m] - max_n vA[m,:]
  stream B: vB[n,m] = 2*g.p - p2[m];  min_m d2[:,n] = g2[n] - max_m vB[n,:]
Matmuls run in float32r (1 PE cycle/row vs 4 for fp32) with a hi/lo split
(x = xh + xl, three exact cross terms) for fp32-level accuracy, and produce
-d2 DIRECTLY (p2 and g2 both folded in, K=13).  Because -d2 is small near
its max (no large-term cancellation), the PSUM tiles are converted to bf16
by the otherwise-idle ScalarE (Relu of -1x, which also clamps rounding
negatives), and the min-reductions run on DVE as packed-bf16 2x-mode
tensor_tensor min cascades from SBUF — about half the cost of 1x fp32
tensor_reduce from PSUM, splitting the reduction load across two engines.
"""

import numpy as np

B, D, M, N = 8, 3, 8192, 8192
P = 128  # partitions / tile size
NCHUNK = 512  # matmul free dim (one PSUM bank of fp32)
GROUP = 4  # chunks per PSUM tile (4 banks -> FD=2048 reductions)
EPS = 1e-8

_built = None


def _build():
    import concourse.bass as bass
    import concourse.bacc as bacc
    import concourse.mybir as mybir
    import concourse.tile as tile
    from contextlib import ExitStack

    fp32 = mybir.dt.float32
    nc = bacc.Bacc()
    p_ext = nc.declare_dram_parameter("p", [D, M], fp32, isOutput=False)
    g_ext = nc.declare_dram_parameter("g", [D, N], fp32, isOutput=False)
    negones_ext = nc.declare_dram_parameter(
        "negones", [1, M], mybir.dt.float32r, isOutput=False
    )
    out_ext = nc.declare_dram_parameter("out", [P, 2], fp32, isOutput=True)

    ntiles = M // P  # 64 (same for N)
    ngroups = N // (GROUP * NCHUNK)  # 4 groups of 2048 per tile row

    with ExitStack() as ctx:
        tc = ctx.enter_context(tile.TileContext(nc))
        singles = ctx.enter_context(tc.tile_pool(name="singles", bufs=1))
        scratch = ctx.enter_context(tc.tile_pool(name="scratch", bufs=1))
        psum = ctx.enter_context(tc.tile_pool(name="psum", bufs=2, space="PSUM"))

        # weights-form (2x,2y,2z,1) and rhs-form (x,y,z,-sq) matrices.
        # float32r: PE runs these at 1 cycle/row for N>=256 (vs 4 for fp32);
        # producers must emit fp32r-rounded values (walrus verifier rule).
        fp32r = mybir.dt.float32r
        bf16 = mybir.dt.bfloat16
        pw = singles.tile([13, M], fp32r)
        pr = singles.tile([13, M], fp32r)
        gw = singles.tile([13, N], fp32r)
        gr = singles.tile([13, N], fp32r)
        # per tile-row: groups 0,1 -> bf16 pair cascade; group 2 -> bf16
        # solo cascade; group 3 -> direct fp32 reduce of -d2 on DVE. This
        # balances ScalarE (3 conversions) vs DVE (cascades + 1 direct).
        rowparts = singles.tile([P, ntiles, 2], fp32)
        colparts = singles.tile([P, ntiles, 2], fp32)
        rowdpart = singles.tile([P, ntiles], fp32)  # max(-d2), direct group
        coldpart = singles.tile([P, ntiles], fp32)
        ones3 = singles.tile([D, P], fp32)
        nc.vector.memset(ones3, 1.0)
        eps_col = singles.tile([P, 1], fp32)
        nc.vector.memset(eps_col, EPS)
        out_sb = singles.tile([P, 2], fp32)

        SPAN = GROUP * NCHUNK  # 2048: chunk width for staged builds

        def setup_side(ext, w, r, nm):
            """Build the K=13 hi/lo matrices.

            w = (2xh[3], 2xh[3], xl[3], -1, -1, sqh, sql)  (weights form)
            r = (xh[3],  xl[3], 2xh[3], sqh, sql, -1, -1)  (rhs form)
            so that sum_k w_k*r'_k = 2 x.y - y2 - x2 = -d2 exactly to ~fp32.
            Engine ops may only start at partition 0, so rows >=3 are
            filled by DMA from partition-0 staging tiles or row dups.
            """
            raw = scratch.tile([D, M], fp32, tag="raw", name=f"raw_{nm}")
            nc.sync.dma_start(out=raw, in_=ext[:, :])
            # hi parts (fp32r writes round): w[0:3]=2xh (exact 2*round(x)),
            # r[0:3]=xh
            nc.vector.tensor_scalar_mul(w[0:3, :], raw, 2.0)
            nc.vector.tensor_copy(r[0:3, :], raw)
            # dup rows: w[3:6]=2xh, r[6:9]=2xh
            nc.sync.dma_start(out=w[3:6, :], in_=w[0:3, :])
            nc.sync.dma_start(out=r[6:9, :], in_=w[0:3, :])
            # const rows: w[9:11] = -1, r[11:13] = -1
            nc.sync.dma_start(out=w[9:10, :], in_=negones_ext[:, :])
            nc.sync.dma_start(out=w[10:11, :], in_=negones_ext[:, :])
            nc.sync.dma_start(out=r[11:12, :], in_=negones_ext[:, :])
            nc.sync.dma_start(out=r[12:13, :], in_=negones_ext[:, :])
            # lo parts per chunk: xl = round(x - xh) -> w[6:9], r[3:6]
            for ci in range(M // SPAN):
                sl = slice(ci * SPAN, (ci + 1) * SPAN)
                st_xl = scratch.tile(
                    [D, SPAN], fp32r, tag="st_xl", bufs=2, name="st_xl"
                )
                nc.vector.tensor_tensor(
                    out=st_xl, in0=raw[:, sl], in1=r[0:3, sl].bitcast(fp32),
                    op=mybir.AluOpType.subtract,
                )
                nc.sync.dma_start(out=w[6:9, sl], in_=st_xl)
                nc.sync.dma_start(out=r[3:6, sl], in_=st_xl)
            nc.vector.tensor_mul(raw, raw, raw)  # squares, in place
            # sq rows split hi/lo at partition 0 (1024-wide chunks to keep
            # the staging slots small), DMA to r rows 9/10 and w rows 11/12
            SQW = 1024
            for gi in range(M // SQW):
                sl = slice(gi * SQW, (gi + 1) * SQW)
                ps = psum.tile([P, GROUP, NCHUNK], fp32, tag="ps", name="ps_sq")
                for c in range(SQW // NCHUNK):
                    n0 = gi * SQW + c * NCHUNK
                    nc.tensor.matmul(
                        ps[:, c, :], ones3, raw[:, n0 : n0 + NCHUNK],
                        start=True, stop=True,
                    )
                psrow = ps[0:1, 0:2, :].rearrange("p a b -> p (a b)")
                st_sqh = scratch.tile([1, SQW], fp32r, tag="st_sqh", name="st_sqh")
                st_sql = scratch.tile([1, SQW], fp32r, tag="st_sql", name="st_sql")
                nc.scalar.copy(st_sqh, psrow)
                nc.vector.tensor_tensor(
                    out=st_sql, in0=psrow, in1=st_sqh.bitcast(fp32),
                    op=mybir.AluOpType.subtract,
                )
                nc.sync.dma_start(out=r[9:10, sl], in_=st_sqh)
                nc.sync.dma_start(out=r[10:11, sl], in_=st_sql)
                nc.sync.dma_start(out=w[11:12, sl], in_=st_sqh)
                nc.sync.dma_start(out=w[12:13, sl], in_=st_sql)

        setup_side(p_ext, pw, pr, "p")
        setup_side(g_ext, gw, gr, "g")

        # --- main loops: two symmetric streams.  PSUM holds -d2; ScalarE
        # converts each 2048-group to bf16 d2 (Relu of -1x: also clamps
        # fp-rounding negatives), two groups fill a [128, 4096] bf16 buffer,
        # then DVE folds it with packed-bf16 2x tensor_tensor mins + a final
        # 1x reduce.  ACT ~1.85us and DVE ~1.3us per group vs 2.26us DVE-only.
        def stream(w, r, parts, dparts):
            for t in range(ntiles):
                lhsT = w[:, t * P : (t + 1) * P]  # [13, 128]
                # --- groups 0,1: pair cascade ---
                bfp = scratch.tile(
                    [P, 2 * SPAN], bf16, tag="st_xl", bufs=2, name="bfp"
                )
                for sub in range(2):
                    ps = psum.tile(
                        [P, GROUP, NCHUNK], fp32, tag="ps", name="ps_main"
                    )
                    for c in range(GROUP):
                        n0 = sub * SPAN + c * NCHUNK
                        nc.tensor.matmul(
                            ps[:, c, :], lhsT, r[:, n0 : n0 + NCHUNK],
                            start=True, stop=True,
                        )
                    nc.scalar.activation(
                        bfp[:, sub * SPAN : (sub + 1) * SPAN],
                        ps.rearrange("p a b -> p (a b)"),
                        mybir.ActivationFunctionType.Relu,
                        scale=-1.0,
                    )
                c1 = scratch.tile([P, SPAN], bf16, tag="st_sqh", name="c1")
                nc.vector.tensor_tensor(
                    out=c1, in0=bfp[:, 0:SPAN], in1=bfp[:, SPAN : 2 * SPAN],
                    op=mybir.AluOpType.min,
                )
                c2 = scratch.tile([P, SPAN // 2], bf16, tag="st_sql", name="c2")
                nc.vector.tensor_tensor(
                    out=c2, in0=c1[:, 0 : SPAN // 2], in1=c1[:, SPAN // 2 :],
                    op=mybir.AluOpType.min,
                )
                c3 = scratch.tile([P, SPAN // 4], bf16, tag="small", name="c3")
                nc.vector.tensor_tensor(
                    out=c3, in0=c2[:, 0 : SPAN // 4], in1=c2[:, SPAN // 4 :],
                    op=mybir.AluOpType.min,
                )
                nc.vector.tensor_reduce(
                    parts[:, t, 0:1], c3,
                    axis=mybir.AxisListType.X, op=mybir.AluOpType.min,
                )
                # --- group 2: solo cascade (bfs shares the bfp slots) ---
                ps = psum.tile([P, GROUP, NCHUNK], fp32, tag="ps", name="ps_m2")
                for c in range(GROUP):
                    n0 = 2 * SPAN + c * NCHUNK
                    nc.tensor.matmul(
                        ps[:, c, :], lhsT, r[:, n0 : n0 + NCHUNK],
                        start=True, stop=True,
                    )
                bfs = scratch.tile(
                    [P, SPAN], bf16, tag="st_xl", bufs=2, name="bfs"
                )
                nc.scalar.activation(
                    bfs, ps.rearrange("p a b -> p (a b)"),
                    mybir.ActivationFunctionType.Relu, scale=-1.0,
                )
                s1 = scratch.tile([P, SPAN // 2], bf16, tag="raw", name="s1")
                nc.vector.tensor_tensor(
                    out=s1, in0=bfs[:, 0 : SPAN // 2], in1=bfs[:, SPAN // 2 :],
                    op=mybir.AluOpType.min,
                )
                s2 = scratch.tile([P, SPAN // 4], bf16, tag="small2", name="s2")
                nc.vector.tensor_tensor(
                    out=s2, in0=s1[:, 0 : SPAN // 4], in1=s1[:, SPAN // 4 :],
                    op=mybir.AluOpType.min,
                )
                nc.vector.tensor_reduce(
                    parts[:, t, 1:2], s2,
                    axis=mybir.AxisListType.X, op=mybir.AluOpType.min,
                )
                # --- group 3: direct fp32 reduce (max of -d2) ---
                ps = psum.tile([P, GROUP, NCHUNK], fp32, tag="ps", name="ps_m3")
                for c in range(GROUP):
                    n0 = 3 * SPAN + c * NCHUNK
                    nc.tensor.matmul(
                        ps[:, c, :], lhsT, r[:, n0 : n0 + NCHUNK],
                        start=True, stop=True,
                    )
                nc.vector.tensor_reduce(
                    dparts[:, t : t + 1], ps,
                    axis=mybir.AxisListType.XY, op=mybir.AluOpType.max,
                )

        # B first: its inputs (gw built early in g-setup, pr from p-setup)
        # are ready before gr's staged sq rows, so PE starts sooner.
        stream(gw, pr, colparts, coldpart)  # d2^T minima
        stream(pw, gr, rowparts, rowdpart)  # d2 minima

        # --- tails: el = sqrt(min over partials + eps), per-partition sums.
        # direct-group partial is max(-d2): negate+clamp, then min in.
        def tail(parts, dparts, col):
            d2m = scratch.tile([P, ntiles], fp32, tag="small2", name=f"d2_{col}")
            nc.vector.tensor_reduce(
                d2m, parts, axis=mybir.AxisListType.X, op=mybir.AluOpType.min
            )
            d2d = scratch.tile([P, ntiles], fp32, tag="small", name=f"d2d_{col}")
            nc.vector.tensor_scalar(
                d2d, dparts, -1.0, 0.0,
                op0=mybir.AluOpType.mult, op1=mybir.AluOpType.max,
            )
            nc.vector.tensor_tensor(
                out=d2m, in0=d2m, in1=d2d, op=mybir.AluOpType.min
            )
            el = scratch.tile([P, ntiles], fp32, tag="small3", name=f"el_{col}")
            nc.scalar.activation(
                el, d2m, mybir.ActivationFunctionType.Sqrt, bias=eps_col
            )
            nc.vector.tensor_reduce(
                out_sb[:, col : col + 1], el,
                axis=mybir.AxisListType.X, op=mybir.AluOpType.add,
            )

        tail(rowparts, rowdpart, 0)
        tail(colparts, coldpart, 1)

        nc.sync.dma_start(out=out_ext[:, :], in_=out_sb)

    nc.finalize()  # Bacc: compile passes (wait splitting, reg alloc) + freeze
    return nc


def _get_nc():
    global _built
    if _built is None:
        _built = _build()
    return _built


def _run(in_maps, **kw):
    from concourse.bass_utils import run_bass_kernel_spmd

    return run_bass_kernel_spmd(_get_nc(), in_maps, list(range(B)), **kw)


_runner = None


def _get_runner():
    """Cached jitted SPMD executable (run_bass_kernel_spmd rebuilds the jax
    trace on every call, ~0.5s; this keeps the compiled callable alive)."""
    global _runner
    if _runner is not None:
        return _runner
    import jax
    import concourse.mybir as mybir
    from jax.experimental.shard_map import shard_map
    from jax.sharding import Mesh, PartitionSpec
    from concourse import bass2jax

    nc = _get_nc()
    bass2jax.install_neuronx_cc_hook()
    partition_name = nc.partition_id_tensor.name if nc.partition_id_tensor else None
    in_names, out_names, out_avals, zero_shapes = [], [], [], []
    for alloc in nc.m.functions[0].allocations:
        if not isinstance(alloc, mybir.MemoryLocationSet):
            continue
        name = alloc.memorylocations[0].name
        if alloc.kind == "ExternalInput":
            if name != partition_name:
                in_names.append(name)
        elif alloc.kind == "ExternalOutput":
            shape = tuple(alloc.tensor_shape)
            dtype = mybir.dt.np(alloc.dtype)
            out_names.append(name)
            out_avals.append(jax.core.ShapedArray(shape, dtype))
            zero_shapes.append((shape, dtype))
    n_params = len(in_names)
    all_names = in_names + out_names + ([partition_name] if partition_name else [])

    def _body(*args):
        operands = list(args)
        if partition_name is not None:
            operands.append(bass2jax.partition_id_tensor())
        return tuple(
            bass2jax._bass_exec_p.bind(
                *operands,
                out_avals=tuple(out_avals),
                in_names=tuple(all_names),
                out_names=tuple(out_names),
                lowering_input_output_aliases=(),
                sim_require_finite=True,
                sim_require_nnan=True,
                nc=nc,
            )
        )

    devices = jax.devices()[:B]
    mesh = Mesh(np.asarray(devices), ("core",))
    donate = tuple(range(n_params, n_params + len(out_names)))
    sharded = jax.jit(
        shard_map(
            _body,
            mesh=mesh,
            in_specs=(PartitionSpec("core"),) * (n_params + len(out_names)),
            out_specs=(PartitionSpec("core"),) * len(out_names),
            check_rep=False,
        ),
        donate_argnums=donate,
        keep_unused=True,
    )
    _runner = (sharded, in_names, out_names, zero_shapes)
    return _runner


def _run_fast(in_maps):
    sharded, in_names, out_names, zero_shapes = _get_runner()
    concat_in = [
        np.concatenate([np.asarray(in_maps[c][nm]) for c in range(B)], axis=0)
        for nm in in_names
    ]
    concat_zeros = [
        np.zeros((B * s[0], *s[1:]), dt) for (s, dt) in zero_shapes
    ]
    out_arrs = sharded(*concat_in, *concat_zeros)
    outs = []
    for c in range(B):
        d = {}
        for i, nm in enumerate(out_names):
            s, dt = zero_shapes[i]
            d[nm] = np.asarray(out_arrs[i]).reshape(B, *s)[c]
        outs.append(d)
    return outs


def kernel(predict_pc, gt_pc):
    predict_pc = np.asarray(predict_pc, dtype=np.float32)
    gt_pc = np.asarray(gt_pc, dtype=np.float32)
    negones = np.full((1, M), -1.0, dtype=np.float32)
    in_maps = [
        {
            "p": np.ascontiguousarray(predict_pc[i]),
            "g": np.ascontiguousarray(gt_pc[i]),
            "negones": negones,
        }
        for i in range(B)
    ]
    results = _run_fast(in_maps)
    total = 0.0
    for r in results:
        o = np.asarray(r["out"], dtype=np.float64)
        total += o[:, 0].sum() / (B * M) + o[:, 1].sum() / (B * N)
    return np.float32(total)
